# revision 45
# baseline (speedup 1.0000x reference)
"""Trainium2 Bass kernel for nn_FIS_ImportanceAssessment.

Reference computation, per pixel (B=16, C=256, H=W=64):
    sumsq = sum_c f^2 ; sum = sum_c f
    mag   = clip(sqrt(sumsq/C), 0, 1)
    var   = clip((sumsq - sum^2/C)/(C-1), 0, 1)
    grad  = sqrt(var_clipped)               (== clip(sqrt(var), 0, 1))
    out   = sigmoid(relu([mag,var,grad] @ W1 + b1) @ W2 + b2)

Sharding: data-parallel over batch, 2 batches per core across 8 cores.

Per-core layout trick: the C-axis reduction is done on the PE with a
"block one-hot" stationary operand (float32r -> single-pass matmuls;
plain fp32 matmuls cost 4 cycles/row).  The core's 8192 pixels are
split into 16 chunks of 512; chunk g's column sums land on PSUM
partitions [8g, 8g+8), i.e. the stats arrive replicated 8x in an
(group g in 0..15, replica oh in 0..7) partition layout.  The replicas
let the whole 3->16->1 MLP run as per-partition tensor_scalar /
scalar_tensor_tensor ops (weight scalars vary only across partitions),
done twice for the two halves of the 16 hidden channels, followed by a
block-diagonal-W2 matmul that contracts the hidden channels.
"""

from contextlib import ExitStack

import numpy as np

import concourse.bacc as bacc
import concourse.bass as bass
import concourse.tile as tile
from concourse import mybir

F32 = mybir.dt.float32
F32R = mybir.dt.float32r  # TF32-style single-pass PE dtype (fp32 is 4 cyc/row)
BF16 = mybir.dt.bfloat16
AF = mybir.ActivationFunctionType
OP = mybir.AluOpType

# -------- problem geometry (hardcoded per contract) --------
B, C, H, W = 16, 256, 64, 64
NCORES = 8
B_PER_CORE = B // NCORES          # 2
PIX = B_PER_CORE * H * W          # 8192 pixels per core
NG = 16                           # pixel chunks ("groups") per core
NREP = 8                          # o-replication factor (128 / NG)
CHUNK = PIX // NG                 # 512 pixels per chunk (= 1 PSUM bank)
NHID = 16                         # MLP hidden width
NPASS = NHID // NREP              # 2 MLP passes over hidden halves

# consts_r (float32r) column layout: matmul stationary operands
ZCOL = 0          # [0:256)   block-one-hot window source (cols 128..135 = 1)
BDCOL = 256       # [256:288) block-diag W2: 256 + k*16 + g
DCOL = 288        # [288:416) diag(-1/C): folds -sum^2/C into psum_sq via PE
NCONST_R = 416
# consts_f (float32) column layout: per-partition scalar operands
#   [0:6) W1 scalars 3*k+i ; [6:8) b1 ; [8] b2
NCONST_F = 16


def build_nc() -> bass.Bass:
    # Bacc (not raw Bass): its finalize() runs generate_event_semaphores,
    # which splits multi-sem waits to satisfy the 1-wait-per-instruction
    # hardware constraint that walrus codegen enforces.
    nc = bacc.Bacc()
    # float32r end-to-end for everything the PE consumes: the BIR verifier
    # requires fp32r-matmul inputs to be *produced* as float32r.
    feat = nc.dram_tensor(
        "features", [B_PER_CORE, C, H * W], F32R, kind="ExternalInput"
    )
    cst_r = nc.dram_tensor("consts_r", [128, NCONST_R], F32R, kind="ExternalInput")
    cst_h = nc.dram_tensor("consts_h", [128, 256], BF16, kind="ExternalInput")
    cst_f = nc.dram_tensor("consts_f", [128, NCONST_F], F32, kind="ExternalInput")
    out_d = nc.dram_tensor("out", [NG, CHUNK], F32, kind="ExternalOutput")

    with tile.TileContext(nc) as tc, ExitStack() as ctx:
        singles = ctx.enter_context(tc.tile_pool(name="singles", bufs=1))
        # bufs=2: both streaming rounds get fresh slots, so no x/sq DMA
        # ever carries a buffer-reuse (WAR) wait on top of its RAW wait.
        xpool = ctx.enter_context(tc.tile_pool(name="xpool", bufs=2))
        sqpool = ctx.enter_context(tc.tile_pool(name="sqpool", bufs=2))
        tailp = ctx.enter_context(tc.tile_pool(name="tailp", bufs=1))
        psump = ctx.enter_context(tc.tile_pool(name="psump", bufs=1, space="PSUM"))

        psum_sum = psump.tile([128, CHUNK], F32)
        psum_sq = psump.tile([128, CHUNK], F32)
        psum2 = psump.tile([NG, CHUNK], F32)

        # Consts first: they're tiny and the very first matmul needs them —
        # queueing them behind feature megabytes stalls the whole PE stream.
        pieces = [(0, 1024), (1024, 1024), (2048, 1024), (3072, 512), (3584, 512)]
        xs, sqs = [], []
        for b in range(B_PER_CORE):
            xs.append(xpool.tile([128, 2, H * W], F32R, tag="x", name=f"x_{b}"))
            sqs.append(sqpool.tile([128, 2, H * W], BF16, tag="sq", name=f"sq_{b}"))

        cons_r = singles.tile([128, NCONST_R], F32R)
        nc.sync.dma_start(out=cons_r, in_=cst_r[:])
        cons_h = singles.tile([128, 256], BF16)
        nc.sync.dma_start(out=cons_h, in_=cst_h[:])
        cons_f = singles.tile([128, NCONST_F], F32)
        nc.sync.dma_start(out=cons_f, in_=cst_f[:])

        # Absorb the consts-DMA waits on the PE here so the first real
        # matmuls only wait on the features/squares. (psum2 is cleared again
        # by the real start=True matmul of the MLP output group later.)
        # (2x2, not 1x1: fp32r matmuls require even free dims.)
        nc.tensor.matmul(
            psum2[0:2, 0:2],
            lhsT=cons_r[:, 0:2],
            rhs=cons_r[:, 0:2],
            start=True,
            stop=True,
        )
        nc.tensor.matmul(
            psum2[0:2, 0:2],
            lhsT=cons_h[:, 0:2],
            rhs=cons_h[:, 0:2],
            start=True,
            stop=True,
        )

        # ---- streaming phase: load, square, PE column-sum reductions ----
        # 1 MiB DMA pieces ([128, 2 C-halves, 1024 px]) so compute starts
        # ~3 us in and stays pipelined with the DMA stream.  Squares cast to
        # bf16 (full PE clock + fast weight load on the squared path) and
        # are spread across ACT/DVE/GPSIMD so no single engine gates the
        # matmul stream.
        # Squares alternate ACT/DVE only — keeping GPSIMD idle avoids its
        # ucode library load in the kernel preamble (a multi-us startup
        # barrier contribution).
        sq_engines = {
            (p, h): ("A" if (p + h) % 2 == 0 else "V")
            for p in range(len(pieces))
            for h in range(2)
        }
        nsum = 0
        nsq = 0
        total_mm = B_PER_CORE * 2 * (H * W // CHUNK)  # 32 per PSUM bank
        for b in range(B_PER_CORE):
            x, sq = xs[b], sqs[b]
            feat_b = feat[b].rearrange("(h c) p -> c h p", h=2)
            for p, (p0, plen) in enumerate(pieces):
                psl = slice(p0, p0 + plen)
                nc.sync.dma_start(out=x[:, :, psl], in_=feat_b[:, :, psl])
                for half in range(2):
                    xin = x[:, half, psl].bitcast(F32)
                    sqo = sq[:, half, psl]
                    if sq_engines[(p, half)] == "A":
                        nc.scalar.activation(sqo, xin, AF.Square)
                    else:
                        nc.vector.tensor_mul(sqo, xin, xin)
                for half in range(2):
                    for q in range(p0 // CHUNK, (p0 + plen) // CHUNK):
                        g = b * (H * W // CHUNK) + q
                        sl = slice(q * CHUNK, (q + 1) * CHUNK)
                        nc.tensor.matmul(
                            psum_sum,
                            lhsT=cons_r[:, 128 - NREP * g : 256 - NREP * g],
                            rhs=x[:, half, sl],
                            start=(nsum == 0),
                            stop=(nsum == total_mm - 1),
                        )
                        nsum += 1
                        nc.tensor.matmul(
                            psum_sq,
                            lhsT=cons_h[:, 128 - NREP * g : 256 - NREP * g],
                            rhs=sq[:, half, sl],
                            start=(nsq == 0),
                            stop=(nsq == total_mm - 1),
                        )
                        nsq += 1

        # ---- stats + MLP tail on (g, oh)-replicated [128, 512] tiles ----
        # The tail is semaphore-latency bound (~0.3-0.5 us per cross-engine
        # hop), so: full-width ops, minimum hop count, mag-branch work runs
        # on DVE while ACT computes grad=sqrt, b1-add + relu fused into one
        # ACT op, k=0/k=1 passes interleaved.
        #
        # NOTE on dropped clamps: u = sumsq - sum^2/C >= 0 by Cauchy-Schwarz;
        # fp error is ~1e-3 abs vs u ~ 150-400 for N(0,1) inputs, so the
        # max(var,0) clamp can never bind and is omitted.
        inv_c = 1.0 / C
        inv_cm1 = 1.0 / (C - 1)

        a = tailp.tile([128, CHUNK], F32)  # sum^2
        nc.scalar.activation(a, psum_sum, AF.Square)
        mag = tailp.tile([128, CHUNK], F32)
        nc.scalar.activation(mag, psum_sq, AF.Sqrt, scale=inv_c)
        u = tailp.tile([128, CHUNK], F32)  # sumsq - sum^2/C
        nc.vector.scalar_tensor_tensor(
            u, in0=a, scalar=-inv_c, in1=psum_sq, op0=OP.mult, op1=OP.add
        )
        var_c = tailp.tile([128, CHUNK], F32)  # clip(var, 0, 1)
        nc.vector.tensor_scalar(
            var_c, in0=u, scalar1=inv_cm1, scalar2=1.0, op0=OP.mult, op1=OP.min
        )
        grad = tailp.tile([128, CHUNK], F32)
        nc.scalar.activation(grad, var_c, AF.Sqrt)

        # MLP, hidden-half passes interleaved.  z = w0*min(mag,1) + w1*var
        # + w2*grad + b1; the mag terms (clip fused into the same op) are
        # computed while ACT produces grad, then two fused adds, then
        # hk = relu(t2 + b1) on ACT (bias AP add is free).
        tms, t1s, t2s, hks = [], [], [], []
        for k in range(NPASS):
            w0 = cons_f[:, 3 * k + 0 : 3 * k + 1]
            tm = tailp.tile([128, CHUNK], F32, name=f"tm_{k}")
            nc.vector.tensor_scalar(
                tm, in0=mag, scalar1=1.0, scalar2=w0, op0=OP.min, op1=OP.mult
            )
            tms.append(tm)
        for k in range(NPASS):
            w1 = cons_f[:, 3 * k + 1 : 3 * k + 2]
            t1 = tailp.tile([128, CHUNK], F32, name=f"t1_{k}")
            nc.vector.scalar_tensor_tensor(
                t1, in0=var_c, scalar=w1, in1=tms[k], op0=OP.mult, op1=OP.add
            )
            t1s.append(t1)
        for k in range(NPASS):
            w2 = cons_f[:, 3 * k + 2 : 3 * k + 3]
            b1c = cons_f[:, 6 + k : 7 + k]
            t2 = tailp.tile([128, CHUNK], F32, name=f"t2_{k}")
            nc.vector.scalar_tensor_tensor(
                t2, in0=grad, scalar=w2, in1=t1s[k], op0=OP.mult, op1=OP.add
            )
            t2s.append(t2)
            # hk = relu(t2 + b1): fused b1-add + relu in one DVE op (keeps
            # the chain on-engine; ACT adds a cross-engine hop).
            hk = tailp.tile([128, CHUNK], F32R, name=f"hk_{k}")
            nc.vector.tensor_scalar(
                hk, in0=t2, scalar1=b1c, scalar2=0.0, op0=OP.add, op1=OP.max
            )
            hks.append(hk)
            nc.tensor.matmul(
                psum2,
                lhsT=cons_r[:, BDCOL + NG * k : BDCOL + NG * (k + 1)],
                rhs=hk,
                start=(k == 0),
                stop=(k == NPASS - 1),
            )

        # Sigmoid + store in two halves so the first output DMA overlaps the
        # second sigmoid.
        out_sb = tailp.tile([NG, CHUNK], F32)
        for c in range(2):
            cs = slice(c * (CHUNK // 2), (c + 1) * (CHUNK // 2))
            nc.scalar.activation(
                out_sb[:, cs], psum2[:, cs], AF.Sigmoid, bias=cons_f[:NG, 8:9]
            )
            nc.sync.dma_start(out=out_d[:, cs], in_=out_sb[:, cs])

    nc.finalize()
    return nc


def make_consts(W1, b1, W2, b2):
    cr = np.zeros((128, NCONST_R), np.float32)
    cr[:, 128 : 128 + NREP] = 1.0  # ones block for the windowed one-hot lhsT
    cr[:, DCOL : DCOL + 128][np.arange(128), np.arange(128)] = -1.0 / C
    ch = np.zeros((128, 256), np.float32)
    ch[:, 128 : 128 + NREP] = 1.0  # same, bf16 flavor for the squared path
    cf = np.zeros((128, NCONST_F), np.float32)
    for g in range(NG):
        for oh in range(NREP):
            p = g * NREP + oh
            for k in range(NPASS):
                o = k * NREP + oh
                for i in range(3):
                    cf[p, k * 3 + i] = W1[i, o]
                cf[p, 6 + k] = b1[o]
                cr[p, BDCOL + k * NG + g] = W2[o, 0]
    cf[:, 8] = b2[0]
    try:
        import ml_dtypes

        ch = ch.astype(ml_dtypes.bfloat16)
    except ImportError:
        ch = ch.astype(np.uint16)  # won't happen: ml_dtypes ships with jax
    return cr, ch, cf


_CACHE: dict = {}


def _get_nc() -> bass.Bass:
    if "nc" not in _CACHE:
        _CACHE["nc"] = build_nc()
    return _CACHE["nc"]


def run_sharded(features, W1, b1, W2, b2, **spmd_kwargs):
    """Run the SPMD kernel; returns (BassKernelResults, assembled output)."""
    from concourse.bass_utils import run_bass_kernel_spmd

    feats = np.ascontiguousarray(features, dtype=np.float32).reshape(B, C, H * W)
    cr, ch, cf = make_consts(
        np.asarray(W1, np.float32),
        np.asarray(b1, np.float32),
        np.asarray(W2, np.float32),
        np.asarray(b2, np.float32),
    )
    in_maps = [
        {
            "features": np.ascontiguousarray(
                feats[r * B_PER_CORE : (r + 1) * B_PER_CORE]
            ),
            "consts_r": cr,
            "consts_h": ch,
            "consts_f": cf,
        }
        for r in range(NCORES)
    ]
    nc = _get_nc()
    res = run_bass_kernel_spmd(nc, in_maps, core_ids=list(range(NCORES)), **spmd_kwargs)
    out = np.concatenate(
        [res.results[r]["out"].reshape(B_PER_CORE, H, W) for r in range(NCORES)],
        axis=0,
    )
    return res, out


def kernel(features, W1, b1, W2, b2):
    _, out = run_sharded(features, W1, b1, W2, b2)
    return out


# revision 50
# speedup vs baseline: 1.0263x; 1.0263x over previous
"""Trainium2 Bass kernel for nn_FIS_ImportanceAssessment.

Reference computation, per pixel (B=16, C=256, H=W=64):
    sumsq = sum_c f^2 ; sum = sum_c f
    mag   = clip(sqrt(sumsq/C), 0, 1)
    var   = clip((sumsq - sum^2/C)/(C-1), 0, 1)
    grad  = sqrt(var_clipped)               (== clip(sqrt(var), 0, 1))
    out   = sigmoid(relu([mag,var,grad] @ W1 + b1) @ W2 + b2)

Sharding: data-parallel over batch, 2 batches per core across 8 cores.

Per-core layout trick: the C-axis reduction is done on the PE with a
"block one-hot" stationary operand (float32r -> single-pass matmuls;
plain fp32 matmuls cost 4 cycles/row).  The core's 8192 pixels are
split into 16 chunks of 512; chunk g's column sums land on PSUM
partitions [8g, 8g+8), i.e. the stats arrive replicated 8x in an
(group g in 0..15, replica oh in 0..7) partition layout.  The replicas
let the whole 3->16->1 MLP run as per-partition tensor_scalar /
scalar_tensor_tensor ops (weight scalars vary only across partitions),
done twice for the two halves of the 16 hidden channels, followed by a
block-diagonal-W2 matmul that contracts the hidden channels.
"""

from contextlib import ExitStack

import numpy as np

import concourse.bacc as bacc
import concourse.bass as bass
import concourse.tile as tile
from concourse import mybir

F32 = mybir.dt.float32
F32R = mybir.dt.float32r  # TF32-style single-pass PE dtype (fp32 is 4 cyc/row)
BF16 = mybir.dt.bfloat16
AF = mybir.ActivationFunctionType
OP = mybir.AluOpType

# -------- problem geometry (hardcoded per contract) --------
B, C, H, W = 16, 256, 64, 64
NCORES = 8
B_PER_CORE = B // NCORES          # 2
PIX = B_PER_CORE * H * W          # 8192 pixels per core
NG = 16                           # pixel chunks ("groups") per core
NREP = 8                          # o-replication factor (128 / NG)
CHUNK = PIX // NG                 # 512 pixels per chunk (= 1 PSUM bank)
NHID = 16                         # MLP hidden width
NPASS = NHID // NREP              # 2 MLP passes over hidden halves

# consts_r (float32r) column layout: matmul stationary operands
ZCOL = 0          # [0:256)   block-one-hot window source (cols 128..135 = 1)
BDCOL = 256       # [256:288) block-diag W2: 256 + k*16 + g
DCOL = 288        # [288:416) diag(-1/C): folds -sum^2/C into psum_sq via PE
NCONST_R = 416
# consts_f (float32) column layout: per-partition scalar operands
#   [0:6) W1 scalars 3*k+i ; [6:8) b1 ; [8] b2
NCONST_F = 16


def build_nc() -> bass.Bass:
    # Bacc (not raw Bass): its finalize() runs generate_event_semaphores,
    # which splits multi-sem waits to satisfy the 1-wait-per-instruction
    # hardware constraint that walrus codegen enforces.
    nc = bacc.Bacc()
    # float32r end-to-end for everything the PE consumes: the BIR verifier
    # requires fp32r-matmul inputs to be *produced* as float32r.
    feat = nc.dram_tensor(
        "features", [B_PER_CORE, C, H * W], F32R, kind="ExternalInput"
    )
    cst_r = nc.dram_tensor("consts_r", [128, NCONST_R], F32R, kind="ExternalInput")
    cst_h = nc.dram_tensor("consts_h", [128, 288], BF16, kind="ExternalInput")
    cst_f = nc.dram_tensor("consts_f", [128, NCONST_F], F32, kind="ExternalInput")
    out_d = nc.dram_tensor("out", [NG, CHUNK], F32, kind="ExternalOutput")

    with tile.TileContext(nc) as tc, ExitStack() as ctx:
        singles = ctx.enter_context(tc.tile_pool(name="singles", bufs=1))
        # bufs=2: both streaming rounds get fresh slots, so no x/sq DMA
        # ever carries a buffer-reuse (WAR) wait on top of its RAW wait.
        xpool = ctx.enter_context(tc.tile_pool(name="xpool", bufs=2))
        sqpool = ctx.enter_context(tc.tile_pool(name="sqpool", bufs=2))
        tailp = ctx.enter_context(tc.tile_pool(name="tailp", bufs=1))
        psump = ctx.enter_context(tc.tile_pool(name="psump", bufs=1, space="PSUM"))

        psum_sum = psump.tile([128, CHUNK], F32)
        psum_sq = psump.tile([128, CHUNK], F32)
        psum2 = psump.tile([NG, CHUNK], F32)

        # Consts first: they're tiny and the very first matmul needs them —
        # queueing them behind feature megabytes stalls the whole PE stream.
        pieces = [(0, 1024), (1024, 1024), (2048, 1024), (3072, 512), (3584, 512)]
        xs, sqs = [], []
        for b in range(B_PER_CORE):
            xs.append(xpool.tile([128, 2, H * W], F32R, tag="x", name=f"x_{b}"))
            sqs.append(sqpool.tile([128, 2, H * W], BF16, tag="sq", name=f"sq_{b}"))

        cons_r = singles.tile([128, NCONST_R], F32R)
        nc.sync.dma_start(out=cons_r, in_=cst_r[:])
        cons_h = singles.tile([128, 288], BF16)
        nc.sync.dma_start(out=cons_h, in_=cst_h[:])
        cons_f = singles.tile([128, NCONST_F], F32)
        nc.sync.dma_start(out=cons_f, in_=cst_f[:])

        # Absorb the consts-DMA waits on the PE here so the first real
        # matmuls only wait on the features/squares. (psum2 is cleared again
        # by the real start=True matmul of the MLP output group later.)
        # (2x2, not 1x1: fp32r matmuls require even free dims.)
        nc.tensor.matmul(
            psum2[0:2, 0:2],
            lhsT=cons_r[:, 0:2],
            rhs=cons_r[:, 0:2],
            start=True,
            stop=True,
        )
        nc.tensor.matmul(
            psum2[0:2, 0:2],
            lhsT=cons_h[:, 0:2],
            rhs=cons_h[:, 0:2],
            start=True,
            stop=True,
        )

        # ---- streaming phase: load, square, PE column-sum reductions ----
        # 1 MiB DMA pieces ([128, 2 C-halves, 1024 px]) so compute starts
        # ~3 us in and stays pipelined with the DMA stream.  Squares cast to
        # bf16 (full PE clock + fast weight load on the squared path) and
        # are spread across ACT/DVE/GPSIMD so no single engine gates the
        # matmul stream.
        # Squares alternate ACT/DVE only — keeping GPSIMD idle avoids its
        # ucode library load in the kernel preamble (a multi-us startup
        # barrier contribution).
        sq_engines = {
            (p, h): ("A" if (p + h) % 2 == 0 else "V")
            for p in range(len(pieces))
            for h in range(2)
        }
        nsum = 0
        nsq = 0
        total_mm = B_PER_CORE * 2 * (H * W // CHUNK)  # 32 per PSUM bank
        for b in range(B_PER_CORE):
            x, sq = xs[b], sqs[b]
            feat_b = feat[b].rearrange("(h c) p -> c h p", h=2)
            for p, (p0, plen) in enumerate(pieces):
                psl = slice(p0, p0 + plen)
                nc.sync.dma_start(out=x[:, :, psl], in_=feat_b[:, :, psl])
                for half in range(2):
                    xin = x[:, half, psl].bitcast(F32)
                    sqo = sq[:, half, psl]
                    if sq_engines[(p, half)] == "A":
                        nc.scalar.activation(sqo, xin, AF.Square)
                    else:
                        nc.vector.tensor_mul(sqo, xin, xin)
                for half in range(2):
                    for q in range(p0 // CHUNK, (p0 + plen) // CHUNK):
                        g = b * (H * W // CHUNK) + q
                        sl = slice(q * CHUNK, (q + 1) * CHUNK)
                        nc.tensor.matmul(
                            psum_sum,
                            lhsT=cons_r[:, 128 - NREP * g : 256 - NREP * g],
                            rhs=x[:, half, sl],
                            start=(nsum == 0),
                            stop=(nsum == total_mm - 1),
                        )
                        nsum += 1
                        nc.tensor.matmul(
                            psum_sq,
                            lhsT=cons_h[:, 128 - NREP * g : 256 - NREP * g],
                            rhs=sq[:, half, sl],
                            start=(nsq == 0),
                            stop=(nsq == total_mm - 1),
                        )
                        nsq += 1

        # ---- stats + MLP tail on (g, oh)-replicated [128, 512] tiles ----
        # The tail is semaphore-latency bound (~0.3-0.5 us per cross-engine
        # hop), so: full-width ops, minimum hop count, mag-branch work runs
        # on DVE while ACT computes grad=sqrt, b1-add + relu fused into one
        # ACT op, k=0/k=1 passes interleaved.
        #
        # NOTE on dropped clamps: u = sumsq - sum^2/C >= 0 by Cauchy-Schwarz;
        # fp error is ~1e-3 abs vs u ~ 150-400 for N(0,1) inputs, so the
        # max(var,0) clamp can never bind and is omitted.
        inv_c = 1.0 / C
        inv_cm1 = 1.0 / (C - 1)

        a = tailp.tile([128, CHUNK], F32)  # sum^2
        nc.scalar.activation(a, psum_sum, AF.Square)
        mag = tailp.tile([128, CHUNK], F32)
        nc.scalar.activation(mag, psum_sq, AF.Sqrt, scale=inv_c)
        u = tailp.tile([128, CHUNK], F32)  # sumsq - sum^2/C
        nc.vector.scalar_tensor_tensor(
            u, in0=a, scalar=-inv_c, in1=psum_sq, op0=OP.mult, op1=OP.add
        )
        var_c = tailp.tile([128, CHUNK], F32)  # clip(var, 0, 1)
        nc.vector.tensor_scalar(
            var_c, in0=u, scalar1=inv_cm1, scalar2=1.0, op0=OP.mult, op1=OP.min
        )
        grad = tailp.tile([128, CHUNK], F32)
        nc.scalar.activation(grad, var_c, AF.Sqrt)

        # MLP, hidden-half passes interleaved.  z = w0*min(mag,1) + w1*var
        # + w2*grad + b1; the mag terms (clip fused into the same op) are
        # computed while ACT produces grad, then two fused adds, then
        # hk = relu(t2 + b1) on ACT (bias AP add is free).
        tms, t1s, t2s, hks = [], [], [], []
        for k in range(NPASS):
            w0 = cons_f[:, 3 * k + 0 : 3 * k + 1]
            tm = tailp.tile([128, CHUNK], F32, name=f"tm_{k}")
            nc.vector.tensor_scalar(
                tm, in0=mag, scalar1=1.0, scalar2=w0, op0=OP.min, op1=OP.mult
            )
            tms.append(tm)
        for k in range(NPASS):
            w1 = cons_f[:, 3 * k + 1 : 3 * k + 2]
            t1 = tailp.tile([128, CHUNK], F32, name=f"t1_{k}")
            nc.vector.scalar_tensor_tensor(
                t1, in0=var_c, scalar=w1, in1=tms[k], op0=OP.mult, op1=OP.add
            )
            t1s.append(t1)
        for k in range(NPASS):
            w2 = cons_f[:, 3 * k + 2 : 3 * k + 3]
            b1c = cons_f[:, 6 + k : 7 + k]
            t2 = tailp.tile([128, CHUNK], F32, name=f"t2_{k}")
            nc.vector.scalar_tensor_tensor(
                t2, in0=grad, scalar=w2, in1=t1s[k], op0=OP.mult, op1=OP.add
            )
            t2s.append(t2)
            # hk = relu(t2 + b1): fused b1-add + relu in one DVE op (keeps
            # the chain on-engine; ACT adds a cross-engine hop).  bf16 so
            # the W2 matmuls run at full bf16 rate.
            hk = tailp.tile([128, CHUNK], BF16, name=f"hk_{k}")
            nc.vector.tensor_scalar(
                hk, in0=t2, scalar1=b1c, scalar2=0.0, op0=OP.add, op1=OP.max
            )
            hks.append(hk)
            nc.tensor.matmul(
                psum2,
                lhsT=cons_h[:, 256 + NG * k : 256 + NG * (k + 1)],
                rhs=hk,
                start=(k == 0),
                stop=(k == NPASS - 1),
            )

        # Sigmoid + store in two halves so the first output DMA overlaps the
        # second sigmoid.
        out_sb = tailp.tile([NG, CHUNK], F32)
        for c in range(2):
            cs = slice(c * (CHUNK // 2), (c + 1) * (CHUNK // 2))
            nc.scalar.activation(
                out_sb[:, cs], psum2[:, cs], AF.Sigmoid, bias=cons_f[:NG, 8:9]
            )
            nc.sync.dma_start(out=out_d[:, cs], in_=out_sb[:, cs])

    nc.finalize()
    return nc


def make_consts(W1, b1, W2, b2):
    cr = np.zeros((128, NCONST_R), np.float32)
    cr[:, 128 : 128 + NREP] = 1.0  # ones block for the windowed one-hot lhsT
    cr[:, DCOL : DCOL + 128][np.arange(128), np.arange(128)] = -1.0 / C
    ch = np.zeros((128, 288), np.float32)
    ch[:, 128 : 128 + NREP] = 1.0  # same, bf16 flavor for the squared path
    cf = np.zeros((128, NCONST_F), np.float32)
    for g in range(NG):
        for oh in range(NREP):
            p = g * NREP + oh
            for k in range(NPASS):
                o = k * NREP + oh
                for i in range(3):
                    cf[p, k * 3 + i] = W1[i, o]
                cf[p, 6 + k] = b1[o]
                cr[p, BDCOL + k * NG + g] = W2[o, 0]
                ch[p, 256 + k * NG + g] = W2[o, 0]
    cf[:, 8] = b2[0]
    try:
        import ml_dtypes

        ch = ch.astype(ml_dtypes.bfloat16)
    except ImportError:
        ch = ch.astype(np.uint16)  # won't happen: ml_dtypes ships with jax
    return cr, ch, cf


_CACHE: dict = {}


def _get_nc() -> bass.Bass:
    if "nc" not in _CACHE:
        _CACHE["nc"] = build_nc()
    return _CACHE["nc"]


def run_sharded(features, W1, b1, W2, b2, **spmd_kwargs):
    """Run the SPMD kernel; returns (BassKernelResults, assembled output)."""
    from concourse.bass_utils import run_bass_kernel_spmd

    feats = np.ascontiguousarray(features, dtype=np.float32).reshape(B, C, H * W)
    cr, ch, cf = make_consts(
        np.asarray(W1, np.float32),
        np.asarray(b1, np.float32),
        np.asarray(W2, np.float32),
        np.asarray(b2, np.float32),
    )
    in_maps = [
        {
            "features": np.ascontiguousarray(
                feats[r * B_PER_CORE : (r + 1) * B_PER_CORE]
            ),
            "consts_r": cr,
            "consts_h": ch,
            "consts_f": cf,
        }
        for r in range(NCORES)
    ]
    nc = _get_nc()
    res = run_bass_kernel_spmd(nc, in_maps, core_ids=list(range(NCORES)), **spmd_kwargs)
    out = np.concatenate(
        [res.results[r]["out"].reshape(B_PER_CORE, H, W) for r in range(NCORES)],
        axis=0,
    )
    return res, out


def kernel(features, W1, b1, W2, b2):
    _, out = run_sharded(features, W1, b1, W2, b2)
    return out
